# revision 52
# baseline (speedup 1.0000x reference)
"""AttentionAggregator Trainium2 kernel (8-core SPMD, data-parallel over nodes).

Reference computation (per node n, K=32 neighbors, D=128, H=32, O=128):
  att(x) = tanh(x @ W1) @ W2
  scores[n,k] = <att(neib[n,k]), att(node[n])>
  ws = softmax_k(scores);  agg[n] = sum_k ws[n,k] * neib[n,k]
  out = relu([node @ W_node, agg @ W_neib])

v3 design (per core: 6400 nodes = 50 supertiles of 128 nodes, processed in
pairs; each supertile = 4096 neighbor rows = 32 chunks of 128 rows, chunk t
row p = neighbor k=p%32 of node 4t+p//32):
  * scores fold: M2 = W2 W2^T on host, so scores[n,k] = u[n,k] . w[n] with
    u = tanh(neib @ W1), w = tanh(node@W1) @ M2.
  * neib shipped fp8_e4m3 in BOTH layouts, supertile-dim INSIDE the
    partition dim so paired loads are one contiguous run per partition:
      nbnat [128, st, CH, 144]: natural rows, col 128 = 1.0 (ones column
        makes Z = sum_k E land in the aggregation PSUM tile), width 144 for
        16B-aligned DoubleRow chunk pairs;
      nbt [128, st, CH*128]: d-major, stationary of the u matmuls.
  * node feats pre-transposed [D, st, 128] bf16 (lhsT of out1, rhs of vT).
  * max-free softmax (tanh bounds |scores|) with deferred normalization:
    exp writes fp8 E directly into the block-diagonal stationary stripes of
    a persistent pre-zeroed wsel ring; 1/Z (DVE divide) scales out2 in the
    final fused relu (tensor_scalar mult+max).
  * aggregation: fp8 DoubleRow matmuls process chunk PAIRS (contraction
    2x128 rows); group pairs share a [64, 129] PSUM tile (partition offsets
    0/32); two [64,128]->[128,64] transposes (shared identity) build aggT.
  * w replication: DRAM roundtrip (write w pair once, strided re-read) then
    a sel4 matmul broadcasts j-groups across partitions.
  * engine split: PE matmuls; ACT tanh/exp; DVE score mult+reduce, 1/Z,
    fused relus; Pool (gpsimd) PSUM->SBUF copies. DMA: nbT+wscr+w4 on the
    SP HWDGE ring, nbnat+nodet+out on the ACT ring, paired and prefetched
    two pairs ahead. Output written bf16, cast to f32 on host.
"""

import sys

sys.path.insert(0, "/opt/trn_rl_repo")

import numpy as np
import ml_dtypes

N, K, D, H, O = 50000, 32, 128, 32, 128
NCORES = 8
GRP = 2               # supertiles per load/store group
ST_FULL = 50          # supertiles per core (even, for pairing)
NODES_ST = 128        # nodes per supertile
CH = 32               # 128-row chunks per supertile
RP = 128              # rows per chunk
NC_FULL = ST_FULL * NODES_ST          # 6400 nodes/core
NPAD = NC_FULL * NCORES               # 51200
NATW = 144            # natural row width: 128 feats + ones col + pad (16B-aligned pairs)

F8 = ml_dtypes.float8_e4m3
BF = ml_dtypes.bfloat16

_module_cache = {}


def _patch_tile_drain():
    """This container's walrus rejects >1 sync-wait on one instruction; spread
    the TileContext tail-drain waits over extra sync nops."""
    from concourse import mybir
    from concourse import tile as tile_mod
    from concourse.tile import TileContext

    if getattr(TileContext, "_drain_patched", False):
        return
    MAXW = 1

    def _drain_and_barrier(self, tick_clock, wait_clock):
        drain_inst = self.nc.sync.drain()
        wait_clock.add_sem_waits(
            drain_inst.ins, tile_mod.ScopedClock({None: tick_clock.global_clock})
        )
        mi = drain_inst.ins
        ws = list(mi.sync_info.on_wait)
        if len(ws) > MAXW:
            mi.sync_info.on_wait = ws[:MAXW]
            rest = ws[MAXW:]
            for i in range(0, len(rest), MAXW):
                nop = self.nc.sync.nop(nofuse=True)
                nmi = nop.ins
                if nmi.sync_info is None:
                    nmi.sync_info = mybir.SyncInfo(
                        on_wait=rest[i : i + MAXW], on_update=[]
                    )
                else:
                    nmi.sync_info.on_wait = rest[i : i + MAXW]
        self.nc.all_engine_barrier()
        assert self.sems is not None
        popped = self.nc._tile_sem_poison_stack.pop()
        assert popped is self._sem_poison
        self.nc.clear_and_free_semaphores(list(self.sems.allocated().values()))
        self.nc.all_engine_barrier()

    TileContext._drain_and_barrier = _drain_and_barrier
    TileContext._drain_patched = True


def _split_multi_waits(nc, maxw=1):
    """Walrus in this container allows only one sync-wait per instruction:
    hoist extra waits onto same-engine NOPs inserted just before."""
    from concourse import mybir

    nsplit = 0
    for f in nc.m.functions:
        for b in f.blocks:
            changed = False
            out = []
            for inst in list(b.instructions):
                si = getattr(inst, "sync_info", None)
                ws = list(si.on_wait) if si is not None and si.on_wait else []
                if len(ws) > maxw:
                    keep = ws[-maxw:]
                    rest = ws[:-maxw]
                    for i in range(0, len(rest), maxw):
                        nop = mybir.InstNoOp(
                            name=f"I-wsplit{nc.next_id()}", ins=[], outs=[]
                        )
                        nop.engine = inst.engine
                        nop.sync_info = mybir.SyncInfo(
                            on_wait=rest[i : i + maxw], on_update=[]
                        )
                        out.append(nop)
                    si.on_wait = keep
                    changed = True
                    nsplit += 1
                out.append(inst)
            if changed:
                b.instructions = out
    return nsplit


def build_module(st=ST_FULL, repeat=1, bufs_bigs=2, bufs_mids=3, bufs_uw=3,
                 bufs_wrep=2, bufs_agg=1, bufs_small=2, split_waits=True):
    import concourse.bass as bass
    from concourse import mybir
    from concourse.tile import TileContext
    from concourse.masks import make_identity

    _patch_tile_drain()
    assert st % GRP == 0

    f32 = mybir.dt.float32
    bf16 = mybir.dt.bfloat16
    f8 = mybir.dt.float8e4
    AF = mybir.ActivationFunctionType
    ALU = mybir.AluOpType
    DR = mybir.MatmulPerfMode.DoubleRow
    ncn = st * NODES_ST
    npair = st // GRP

    nc = bass.Bass()
    nodet = nc.declare_dram_parameter("nodet", [D, st, NODES_ST], bf16, isOutput=False)
    nbnat = nc.declare_dram_parameter("nbnat", [RP, st, CH, NATW], f8, isOutput=False)
    nbt = nc.declare_dram_parameter("nbt", [D, st, CH * RP], f8, isOutput=False)
    w1b = nc.declare_dram_parameter("w1b", [D, H], bf16, isOutput=False)
    w1q = nc.declare_dram_parameter("w1q", [D, H], f8, isOutput=False)
    m2b = nc.declare_dram_parameter("m2b", [H, H], bf16, isOutput=False)
    wnodeb = nc.declare_dram_parameter("wnodeb", [D, O], bf16, isOutput=False)
    wneibb = nc.declare_dram_parameter("wneibb", [D, O], bf16, isOutput=False)
    sel4p = nc.declare_dram_parameter("sel4", [4, 128], bf16, isOutput=False)
    out = nc.declare_dram_parameter("out", [ncn, 2 * O], bf16, isOutput=True)
    wscr = nc.dram_tensor("wscr", [st, NODES_ST, H], bf16)

    with TileContext(nc) as tc:
        with (
            tc.tile_pool(name="singles", bufs=1) as singles,
            tc.tile_pool(name="nodep", bufs=3) as nodep,
            tc.tile_pool(name="bigs", bufs=bufs_bigs) as bigs,
            tc.tile_pool(name="mids", bufs=bufs_mids) as mids,
            tc.tile_pool(name="outs", bufs=3) as outs,
            tc.tile_pool(name="ps_uw", bufs=bufs_uw, space="PSUM") as ps_uw,
            tc.tile_pool(name="ps_wrep", bufs=bufs_wrep, space="PSUM") as ps_wrep,
            tc.tile_pool(name="ps_agg", bufs=bufs_agg, space="PSUM") as ps_agg,
            tc.tile_pool(name="ps_small", bufs=bufs_small, space="PSUM") as ps_small,
        ):
            # ---- one-time constants ----
            ident128 = singles.tile([128, 128], f32)
            make_identity(nc, ident128)
            w1b_sb = singles.tile([D, H], bf16)
            nc.gpsimd.dma_start(out=w1b_sb, in_=w1b[:, :])
            w1q_sb = singles.tile([D, H], f8)
            nc.gpsimd.dma_start(out=w1q_sb, in_=w1q[:, :])
            m2b_sb = singles.tile([H, H], bf16)
            nc.gpsimd.dma_start(out=m2b_sb, in_=m2b[:, :])
            wnodeb_sb = singles.tile([D, O], bf16)
            nc.gpsimd.dma_start(out=wnodeb_sb, in_=wnodeb[:, :])
            wneibb_sb = singles.tile([D, O], bf16)
            nc.gpsimd.dma_start(out=wneibb_sb, in_=wneibb[:, :])
            sel4 = singles.tile([4, 128], bf16)
            nc.gpsimd.dma_start(out=sel4, in_=sel4p[:, :])
            ones128 = singles.tile([128, 1], f32)
            nc.vector.memset(ones128, 1.0)
            # wsel ring: persistent pre-zeroed block-diagonal stationaries;
            # only the diagonal stripes are rewritten (by exp) each supertile.
            NW = 4
            wsel_bufs = []
            for i in range(NW):
                wb = singles.tile([128, CH, 128], f8, tag=f"wselring{i}")
                nc.vector.memset(wb, 0.0)
                wsel_bufs.append(wb)

            pair_tiles = {}
            out_tiles = {}
            w4_tiles = {}

            def load_pair(ip):
                """Prefetch both supertiles of pair ip (two iterations ahead).
                nbT rides the SP HWDGE ring; nbnat/nodet the ACT ring."""
                s0 = ip * GRP
                nbT2 = bigs.tile([128, GRP, CH * RP], f8, tag="nbT")
                nc.sync.dma_start(out=nbT2, in_=nbt[:, s0 : s0 + GRP, :])
                nb2 = bigs.tile([128, GRP, CH, NATW], f8, tag="nb")
                nc.scalar.dma_start(out=nb2, in_=nbnat[:, s0 : s0 + GRP, :, :])
                ndT2 = nodep.tile([128, GRP, NODES_ST], bf16, tag="ndT")
                nc.gpsimd.dma_start(out=ndT2, in_=nodet[:, s0 : s0 + GRP, :])
                pair_tiles[ip] = (nbT2, nb2, ndT2)

            def node_pair(ip):
                """out[:,0:128] = relu(node @ W_node); w = tanh(node@W1)@M2
                for both supertiles of the pair; one wscr write."""
                s0 = ip * GRP
                ndT2 = pair_tiles[ip][2]
                w_sb2 = nodep.tile([128, GRP, H], bf16, tag="w_sb")
                out_sb2 = outs.tile([128, GRP, 2 * O], bf16, tag="out_sb")
                out_tiles[ip] = out_sb2
                for g in range(GRP):
                    ndT = ndT2[:, g, :]
                    out1_ps = ps_small.tile([128, O], f32, tag="small")
                    nc.tensor.matmul(out1_ps, lhsT=ndT, rhs=wnodeb_sb)
                    nc.scalar.activation(out_sb2[:, g, 0:O], out1_ps, AF.Relu)
                    vT_ps = ps_small.tile([H, 128], f32, tag="small")
                    nc.tensor.matmul(vT_ps, lhsT=w1b_sb, rhs=ndT)
                    vT_sb = nodep.tile([H, 128], bf16, tag="vT_sb")
                    nc.scalar.activation(vT_sb, vT_ps, AF.Tanh)
                    w_ps = ps_small.tile([128, H], f32, tag="small")
                    nc.tensor.matmul(w_ps, lhsT=vT_sb, rhs=m2b_sb)
                    nc.scalar.copy(w_sb2[:, g, :], w_ps)
                nc.sync.dma_start(
                    out=wscr[s0 : s0 + GRP, :, :].rearrange("s n h -> n s h"),
                    in_=w_sb2,
                )

            def main_open(ip):
                """Pair-level main-path prologue: the strided w re-read."""
                s0 = ip * GRP
                w4 = mids.tile([4, GRP * CH, H], bf16, tag="w4")
                base = wscr[s0 : s0 + 1, 0:1, 0:1]
                in_ap = bass.AP(
                    tensor=base.tensor,
                    offset=base.offset,
                    ap=[[H, 4], [4 * H, GRP * CH], [1, H]],
                )
                nc.sync.dma_start(out=w4, in_=in_ap)
                w4_tiles[ip] = w4

            def main_path(s):
                """neighbor attention + aggregation for supertile s."""
                ip, g = divmod(s, GRP)
                nbT2, nb2, _ = pair_tiles[ip]
                nbT = nbT2[:, g, :]
                nb = nb2[:, g, :, :]
                w4 = w4_tiles[ip]
                # wrep[32j+k, t, h] = w[4t+j, h] via sel4 broadcast matmul
                w4f = w4[:, :, :].rearrange("j t h -> j (t h)")
                wrep_sb = []
                for hh in range(2):
                    wps = ps_wrep.tile([128, 512], f32, tag="wrep")
                    nc.tensor.matmul(
                        wps,
                        lhsT=sel4,
                        rhs=w4f[:, 1024 * g + 512 * hh : 1024 * g + 512 * (hh + 1)],
                    )
                    wrep_sb.append(wps)
                # u = tanh(neib @ W1) natural; tmp = u * wrep; the two
                # halves run on DVE and Pool so they overlap, each with its
                # own per-half reduce to shorten the chain.
                tmp = mids.tile([128, CH, H], bf16, tag="tmp")
                scores = mids.tile([128, CH], f32, tag="scores")
                for half in range(2):
                    u_ps = ps_uw.tile([128, 16 * H], f32, tag="uw")
                    for tt in range(16):
                        t = 16 * half + tt
                        nc.tensor.matmul(
                            u_ps[:, tt * H : (tt + 1) * H],
                            lhsT=nbT[:, t * RP : (t + 1) * RP],
                            rhs=w1q_sb,
                        )
                    u_sb = mids.tile([128, 16, H], bf16, tag="u")
                    nc.scalar.activation(
                        u_sb[:, :, :].rearrange("p t h -> p (t h)"),
                        u_ps[:, :],
                        AF.Tanh,
                    )
                    nc.vector.tensor_mul(
                        tmp[:, 16 * half : 16 * (half + 1), :],
                        u_sb,
                        wrep_sb[half][:, :].rearrange("p (t h) -> p t h", h=H),
                    )
                    nc.vector.tensor_reduce(
                        scores[:, 16 * half : 16 * (half + 1)],
                        tmp[:, 16 * half : 16 * (half + 1), :],
                        axis=mybir.AxisListType.X,
                        op=ALU.add,
                    )
                # exp writes straight into the block-diagonal stationary:
                # wsel[32j+k, 8gg+tm, 4tm+j] = exp(scores[32j+k, 8gg+tm]);
                # element offset 256gg + 36tm + j is a linear-strided AP.
                wsel = wsel_bufs[s % NW]
                w_ap = wsel[:, :, :]
                pstride = w_ap.ap[0][0]
                for j in range(4):
                    stripe = bass.AP(
                        tensor=w_ap.tensor,
                        offset=w_ap.offset + 32 * j * pstride + j,
                        ap=[[pstride, 32], [132, 32]],
                    )
                    sc = scores[32 * j : 32 * (j + 1), :]
                    nc.scalar.activation(stripe, sc, AF.Exp)
                # aggregation: the block-diagonal stationary spans all 128
                # nodes (column 4t+j), so 16 fp8 DoubleRow chunk-pair matmuls
                # accumulate ONE [128, 129] PSUM tile at partition offset 0
                # (DoubleRow requires base partition 0; col 128 = Z).
                rzall = mids.tile([128, 1], f32, tag="rz")
                agg_ps = ps_agg.tile([128, NATW], f32, tag="agg")
                for tm2 in range(16):
                    t = 2 * tm2
                    nc.tensor.matmul(
                        agg_ps[:, 0 : D + 1],
                        lhsT=wsel[:, t : t + 2, :],
                        rhs=nb[:, t : t + 2, 0 : D + 1],
                        start=(tm2 == 0),
                        stop=(tm2 == 15),
                        perf_mode=DR,
                    )
                nc.vector.reciprocal(rzall, agg_ps[:, D : D + 1])
                agg_sb = mids.tile([128, D], f32, tag="agg_sb")
                nc.vector.tensor_copy(agg_sb, agg_ps[:, 0:D])
                aggT_ps = ps_small.tile([128, 128], f32, tag="small")
                nc.tensor.transpose(aggT_ps, agg_sb, ident128)
                aggT_sb = mids.tile([128, 128], bf16, tag="aggT_sb")
                nc.vector.tensor_copy(aggT_sb, aggT_ps)
                out2_ps = ps_small.tile([128, O], f32, tag="small")
                nc.tensor.matmul(out2_ps, lhsT=aggT_sb, rhs=wneibb_sb)
                out_sb2 = out_tiles[ip]
                # relu(x) * (1/Z): fused multiply + max on DVE
                nc.scalar.activation(
                    out_sb2[:, g, O : 2 * O], out2_ps, AF.Relu, scale=rzall[:, 0:1]
                )
                if g == GRP - 1:
                    s0 = ip * GRP
                    nc.gpsimd.dma_start(
                        out=out[s0 * 128 : (s0 + GRP) * 128, :].rearrange(
                            "(s p) o -> p s o", s=GRP
                        ),
                        in_=out_sb2,
                    )
                    pair_tiles.pop(ip)
                    w4_tiles.pop(ip)

            for _rep in range(repeat):
                for i in range(npair + 2):
                    if 1 <= i <= npair:
                        node_pair(i - 1)
                    if i >= 2:
                        main_open(i - 2)
                    if i < npair:
                        load_pair(i)
                    if i >= 2:
                        for g in range(GRP):
                            main_path((i - 2) * GRP + g)

    if split_waits:
        _split_multi_waits(nc)
    return nc


def _host_prep(node_feats, neib_feats, W_att1, W_att2, W_node, W_neib, st=ST_FULL):
    """Per-core input dicts with host-side layouts (supertile dim INSIDE the
    partition dim so paired loads are contiguous per partition):
    nbnat[32j+k, s, t, :] = neib row of (node 4t+j, k), col 128 = 1.0 (fp8);
    nbt[d, s, 128t+(32j+k)] = same rows d-major (fp8);
    nodet[:, s, n] = node_feats[s*128+n, :] (bf16)."""
    W1 = np.ascontiguousarray(np.asarray(W_att1, dtype=np.float32))
    W2 = np.asarray(W_att2, dtype=np.float32)
    M2 = (W2.astype(np.float64) @ W2.astype(np.float64).T).astype(np.float32)
    ncn = st * NODES_ST
    n = node_feats.shape[0]
    npad = ncn * NCORES

    from concurrent.futures import ThreadPoolExecutor

    node_pad = np.zeros((npad, D), dtype=BF)
    node_pad[:n] = np.asarray(node_feats, dtype=np.float32).astype(BF)
    neib_q = np.zeros((npad * K, D), dtype=F8)
    nf32 = np.asarray(neib_feats, dtype=np.float32)
    nrows = n * K
    csz = (nrows + 15) // 16
    with ThreadPoolExecutor(16) as tp:
        list(tp.map(
            lambda i: neib_q[i * csz : min((i + 1) * csz, nrows)].__setitem__(
                slice(None), nf32[i * csz : min((i + 1) * csz, nrows)].astype(F8)
            ),
            range(16),
        ))

    w1b = W1.astype(BF)
    w1q = W1.astype(F8)
    m2b = M2.astype(BF)
    wnodeb = np.ascontiguousarray(np.asarray(W_node, np.float32)).astype(BF)
    wneibb = np.ascontiguousarray(np.asarray(W_neib, np.float32)).astype(BF)
    sel4 = np.zeros((4, 128), dtype=BF)
    for j in range(4):
        sel4[j, 32 * j : 32 * (j + 1)] = 1.0

    def prep_core(c):
        nodes = node_pad[c * ncn : (c + 1) * ncn]
        nodet = np.ascontiguousarray(
            nodes.reshape(st, NODES_ST, D).transpose(2, 0, 1)
        )
        x = neib_q[c * ncn * K : (c + 1) * ncn * K].reshape(st, 32, 4, K, D)
        # y[s, 32j+k, t, d] = x[s, t, j, k, d]
        y = np.ascontiguousarray(x.transpose(0, 2, 3, 1, 4)).reshape(
            st, RP, CH, D
        )
        nbnat = np.zeros((RP, st, CH, NATW), dtype=F8)
        nbnat[:, :, :, 0:D] = y.transpose(1, 0, 2, 3)
        nbnat[:, :, :, D] = np.float32(1.0)
        # nbt[d, s, 128t + p] = y[s, p, t, d]
        nbt = np.ascontiguousarray(y.transpose(3, 0, 2, 1)).reshape(
            D, st, CH * RP
        )
        return {
            "nodet": nodet,
            "nbnat": nbnat,
            "nbt": nbt,
            "w1b": w1b,
            "w1q": w1q,
            "m2b": m2b,
            "wnodeb": wnodeb,
            "wneibb": wneibb,
            "sel4": sel4,
        }

    with ThreadPoolExecutor(NCORES) as tp:
        ins = list(tp.map(prep_core, range(NCORES)))
    return ins


def kernel(node_feats, neib_feats, node_ids, neib_ids, W_att1, W_att2, W_node, W_neib):
    from concourse.bass_utils import run_bass_kernel_spmd

    n = node_feats.shape[0]
    in_maps = _host_prep(node_feats, neib_feats, W_att1, W_att2, W_node, W_neib)

    if "nc" not in _module_cache:
        _module_cache["nc"] = build_module(ST_FULL)
    nc = _module_cache["nc"]

    res = run_bass_kernel_spmd(nc, in_maps, core_ids=list(range(NCORES)))
    outs = np.concatenate([res.results[c]["out"] for c in range(NCORES)], axis=0)
    return np.ascontiguousarray(outs[:n].astype(np.float32))


# revision 54
# speedup vs baseline: 1.0361x; 1.0361x over previous
"""AttentionAggregator Trainium2 kernel (8-core SPMD, data-parallel over nodes).

Reference computation (per node n, K=32 neighbors, D=128, H=32, O=128):
  att(x) = tanh(x @ W1) @ W2
  scores[n,k] = <att(neib[n,k]), att(node[n])>
  ws = softmax_k(scores);  agg[n] = sum_k ws[n,k] * neib[n,k]
  out = relu([node @ W_node, agg @ W_neib])

v3 design (per core: 6400 nodes = 50 supertiles of 128 nodes, processed in
pairs; each supertile = 4096 neighbor rows = 32 chunks of 128 rows, chunk t
row p = neighbor k=p%32 of node 4t+p//32):
  * scores fold: M2 = W2 W2^T on host, so scores[n,k] = u[n,k] . w[n] with
    u = tanh(neib @ W1), w = tanh(node@W1) @ M2.
  * neib shipped fp8_e4m3 in BOTH layouts, supertile-dim INSIDE the
    partition dim so paired loads are one contiguous run per partition:
      nbnat [128, st, CH, 144]: natural rows, col 128 = 1.0 (ones column
        makes Z = sum_k E land in the aggregation PSUM tile), width 144 for
        16B-aligned DoubleRow chunk pairs;
      nbt [128, st, CH*128]: d-major, stationary of the u matmuls.
  * node feats pre-transposed [D, st, 128] bf16 (lhsT of out1, rhs of vT).
  * max-free softmax (tanh bounds |scores|) with deferred normalization:
    exp writes fp8 E directly into the block-diagonal stationary stripes of
    a persistent pre-zeroed wsel ring; 1/Z (DVE divide) scales out2 in the
    final fused relu (tensor_scalar mult+max).
  * aggregation: fp8 DoubleRow matmuls process chunk PAIRS (contraction
    2x128 rows); group pairs share a [64, 129] PSUM tile (partition offsets
    0/32); two [64,128]->[128,64] transposes (shared identity) build aggT.
  * w replication: DRAM roundtrip (write w pair once, strided re-read) then
    a sel4 matmul broadcasts j-groups across partitions.
  * engine split: PE matmuls; ACT tanh/exp; DVE score mult+reduce, 1/Z,
    fused relus; Pool (gpsimd) PSUM->SBUF copies. DMA: nbT+wscr+w4 on the
    SP HWDGE ring, nbnat+nodet+out on the ACT ring, paired and prefetched
    two pairs ahead. Output written bf16, cast to f32 on host.
"""

import sys

sys.path.insert(0, "/opt/trn_rl_repo")

import numpy as np
import ml_dtypes

N, K, D, H, O = 50000, 32, 128, 32, 128
NCORES = 8
GRP = 2               # supertiles per load/store group
ST_FULL = 50          # supertiles per core (even, for pairing)
NODES_ST = 128        # nodes per supertile
CH = 32               # 128-row chunks per supertile
RP = 128              # rows per chunk
NC_FULL = ST_FULL * NODES_ST          # 6400 nodes/core
NPAD = NC_FULL * NCORES               # 51200
NATW = 144            # natural row width: 128 feats + ones col + pad (16B-aligned pairs)

F8 = ml_dtypes.float8_e4m3
BF = ml_dtypes.bfloat16

_module_cache = {}


def _patch_tile_drain():
    """This container's walrus rejects >1 sync-wait on one instruction; spread
    the TileContext tail-drain waits over extra sync nops."""
    from concourse import mybir
    from concourse import tile as tile_mod
    from concourse.tile import TileContext

    if getattr(TileContext, "_drain_patched", False):
        return
    MAXW = 1

    def _drain_and_barrier(self, tick_clock, wait_clock):
        drain_inst = self.nc.sync.drain()
        wait_clock.add_sem_waits(
            drain_inst.ins, tile_mod.ScopedClock({None: tick_clock.global_clock})
        )
        mi = drain_inst.ins
        ws = list(mi.sync_info.on_wait)
        if len(ws) > MAXW:
            mi.sync_info.on_wait = ws[:MAXW]
            rest = ws[MAXW:]
            for i in range(0, len(rest), MAXW):
                nop = self.nc.sync.nop(nofuse=True)
                nmi = nop.ins
                if nmi.sync_info is None:
                    nmi.sync_info = mybir.SyncInfo(
                        on_wait=rest[i : i + MAXW], on_update=[]
                    )
                else:
                    nmi.sync_info.on_wait = rest[i : i + MAXW]
        self.nc.all_engine_barrier()
        assert self.sems is not None
        popped = self.nc._tile_sem_poison_stack.pop()
        assert popped is self._sem_poison
        self.nc.clear_and_free_semaphores(list(self.sems.allocated().values()))
        self.nc.all_engine_barrier()

    TileContext._drain_and_barrier = _drain_and_barrier
    TileContext._drain_patched = True


def _split_multi_waits(nc, maxw=1):
    """Walrus in this container allows only one sync-wait per instruction:
    hoist extra waits onto same-engine NOPs inserted just before."""
    from concourse import mybir

    nsplit = 0
    for f in nc.m.functions:
        for b in f.blocks:
            changed = False
            out = []
            for inst in list(b.instructions):
                si = getattr(inst, "sync_info", None)
                ws = list(si.on_wait) if si is not None and si.on_wait else []
                if len(ws) > maxw:
                    keep = ws[-maxw:]
                    rest = ws[:-maxw]
                    for i in range(0, len(rest), maxw):
                        nop = mybir.InstNoOp(
                            name=f"I-wsplit{nc.next_id()}", ins=[], outs=[]
                        )
                        nop.engine = inst.engine
                        nop.sync_info = mybir.SyncInfo(
                            on_wait=rest[i : i + maxw], on_update=[]
                        )
                        out.append(nop)
                    si.on_wait = keep
                    changed = True
                    nsplit += 1
                out.append(inst)
            if changed:
                b.instructions = out
    return nsplit


def build_module(st=ST_FULL, repeat=1, bufs_bigs=2, bufs_mids=3, bufs_uw=3,
                 bufs_wrep=2, bufs_agg=1, bufs_small=2, split_waits=True):
    import concourse.bass as bass
    from concourse import mybir
    from concourse.tile import TileContext
    from concourse.masks import make_identity

    _patch_tile_drain()
    assert st % GRP == 0

    f32 = mybir.dt.float32
    bf16 = mybir.dt.bfloat16
    f8 = mybir.dt.float8e4
    AF = mybir.ActivationFunctionType
    ALU = mybir.AluOpType
    DR = mybir.MatmulPerfMode.DoubleRow
    ncn = st * NODES_ST
    npair = st // GRP

    nc = bass.Bass()
    nodet = nc.declare_dram_parameter("nodet", [D, st, NODES_ST], bf16, isOutput=False)
    nbnat = nc.declare_dram_parameter("nbnat", [RP, st, CH, NATW], f8, isOutput=False)
    nbt = nc.declare_dram_parameter("nbt", [D, st, CH * RP], f8, isOutput=False)
    w1b = nc.declare_dram_parameter("w1b", [D, H], bf16, isOutput=False)
    w1q = nc.declare_dram_parameter("w1q", [D, H], f8, isOutput=False)
    m2b = nc.declare_dram_parameter("m2b", [H, H], bf16, isOutput=False)
    wnodeb = nc.declare_dram_parameter("wnodeb", [D, O], bf16, isOutput=False)
    wneibb = nc.declare_dram_parameter("wneibb", [D, O], bf16, isOutput=False)
    sel4p = nc.declare_dram_parameter("sel4", [4, 128], bf16, isOutput=False)
    out = nc.declare_dram_parameter("out", [ncn, 2 * O], bf16, isOutput=True)
    wscr = nc.dram_tensor("wscr", [st, NODES_ST, H], bf16)

    with TileContext(nc) as tc:
        with (
            tc.tile_pool(name="singles", bufs=1) as singles,
            tc.tile_pool(name="nodep", bufs=3) as nodep,
            tc.tile_pool(name="bigs", bufs=bufs_bigs) as bigs,
            tc.tile_pool(name="mids", bufs=bufs_mids) as mids,
            tc.tile_pool(name="outs", bufs=3) as outs,
            tc.tile_pool(name="ps_uw", bufs=bufs_uw, space="PSUM") as ps_uw,
            tc.tile_pool(name="ps_wrep", bufs=bufs_wrep, space="PSUM") as ps_wrep,
            tc.tile_pool(name="ps_agg", bufs=bufs_agg, space="PSUM") as ps_agg,
            tc.tile_pool(name="ps_small", bufs=bufs_small, space="PSUM") as ps_small,
        ):
            # ---- one-time constants ----
            ident128 = singles.tile([128, 128], f32)
            make_identity(nc, ident128)
            w1b_sb = singles.tile([D, H], bf16)
            nc.gpsimd.dma_start(out=w1b_sb, in_=w1b[:, :])
            w1q_sb = singles.tile([D, H], f8)
            nc.gpsimd.dma_start(out=w1q_sb, in_=w1q[:, :])
            m2b_sb = singles.tile([H, H], bf16)
            nc.gpsimd.dma_start(out=m2b_sb, in_=m2b[:, :])
            wnodeb_sb = singles.tile([D, O], bf16)
            nc.gpsimd.dma_start(out=wnodeb_sb, in_=wnodeb[:, :])
            wneibb_sb = singles.tile([D, O], bf16)
            nc.gpsimd.dma_start(out=wneibb_sb, in_=wneibb[:, :])
            sel4 = singles.tile([4, 128], bf16)
            nc.gpsimd.dma_start(out=sel4, in_=sel4p[:, :])
            ones128 = singles.tile([128, 1], f32)
            nc.vector.memset(ones128, 1.0)
            # wsel ring: persistent pre-zeroed block-diagonal stationaries;
            # only the diagonal stripes are rewritten (by exp) each supertile.
            NW = 4
            wsel_bufs = []
            for i in range(NW):
                wb = singles.tile([128, CH, 128], f8, tag=f"wselring{i}")
                nc.vector.memset(wb, 0.0)
                wsel_bufs.append(wb)

            pair_tiles = {}
            out_tiles = {}
            w4_tiles = {}

            def load_pair(ip):
                """Prefetch both supertiles of pair ip (two iterations ahead).
                nbT rides the SP HWDGE ring; nbnat/nodet the ACT ring."""
                s0 = ip * GRP
                nbT2 = bigs.tile([128, GRP, CH * RP], f8, tag="nbT")
                nc.sync.dma_start(out=nbT2, in_=nbt[:, s0 : s0 + GRP, :])
                nb2 = bigs.tile([128, GRP, CH, NATW], f8, tag="nb")
                nc.scalar.dma_start(out=nb2, in_=nbnat[:, s0 : s0 + GRP, :, :])
                ndT2 = nodep.tile([128, GRP, NODES_ST], bf16, tag="ndT")
                nc.gpsimd.dma_start(out=ndT2, in_=nodet[:, s0 : s0 + GRP, :])
                pair_tiles[ip] = (nbT2, nb2, ndT2)

            def node_pair(ip):
                """out[:,0:128] = relu(node @ W_node); w = tanh(node@W1)@M2
                for both supertiles of the pair; one wscr write."""
                s0 = ip * GRP
                ndT2 = pair_tiles[ip][2]
                w_sb2 = nodep.tile([128, GRP, H], bf16, tag="w_sb")
                out_sb2 = outs.tile([128, GRP, 2 * O], bf16, tag="out_sb")
                out_tiles[ip] = out_sb2
                for g in range(GRP):
                    ndT = ndT2[:, g, :]
                    out1_ps = ps_small.tile([128, O], f32, tag="small")
                    nc.tensor.matmul(out1_ps, lhsT=ndT, rhs=wnodeb_sb)
                    nc.scalar.activation(out_sb2[:, g, 0:O], out1_ps, AF.Relu)
                    vT_ps = ps_small.tile([H, 128], f32, tag="small")
                    nc.tensor.matmul(vT_ps, lhsT=w1b_sb, rhs=ndT)
                    vT_sb = nodep.tile([H, 128], bf16, tag="vT_sb")
                    nc.scalar.activation(vT_sb, vT_ps, AF.Tanh)
                    w_ps = ps_small.tile([128, H], f32, tag="small")
                    nc.tensor.matmul(w_ps, lhsT=vT_sb, rhs=m2b_sb)
                    nc.scalar.copy(w_sb2[:, g, :], w_ps)
                nc.sync.dma_start(
                    out=wscr[s0 : s0 + GRP, :, :].rearrange("s n h -> n s h"),
                    in_=w_sb2,
                )

            def main_open(ip):
                """Pair-level main-path prologue: the strided w re-read."""
                s0 = ip * GRP
                w4 = mids.tile([4, GRP * CH, H], bf16, tag="w4")
                base = wscr[s0 : s0 + 1, 0:1, 0:1]
                in_ap = bass.AP(
                    tensor=base.tensor,
                    offset=base.offset,
                    ap=[[H, 4], [4 * H, GRP * CH], [1, H]],
                )
                nc.sync.dma_start(out=w4, in_=in_ap)
                w4_tiles[ip] = w4

            def main_path(s):
                """neighbor attention + aggregation for supertile s."""
                ip, g = divmod(s, GRP)
                nbT2, nb2, _ = pair_tiles[ip]
                nbT = nbT2[:, g, :]
                nb = nb2[:, g, :, :]
                w4 = w4_tiles[ip]
                # wrep[32j+k, t, h] = w[4t+j, h] via sel4 broadcast matmul
                w4f = w4[:, :, :].rearrange("j t h -> j (t h)")
                wrep_sb = []
                for hh in range(2):
                    wps = ps_wrep.tile([128, 512], f32, tag="wrep")
                    nc.tensor.matmul(
                        wps,
                        lhsT=sel4,
                        rhs=w4f[:, 1024 * g + 512 * hh : 1024 * g + 512 * (hh + 1)],
                    )
                    wrep_sb.append(wps)
                # u = tanh(neib @ W1) natural; tmp = u * wrep; the two
                # halves run on DVE and Pool so they overlap, each with its
                # own per-half reduce to shorten the chain.
                tmp = mids.tile([128, CH, H], bf16, tag="tmp")
                scores = mids.tile([128, CH], f32, tag="scores")
                for half in range(2):
                    u_ps = ps_uw.tile([128, 16 * H], f32, tag="uw")
                    for tt in range(16):
                        t = 16 * half + tt
                        nc.tensor.matmul(
                            u_ps[:, tt * H : (tt + 1) * H],
                            lhsT=nbT[:, t * RP : (t + 1) * RP],
                            rhs=w1q_sb,
                        )
                    u_sb = mids.tile([128, 16, H], bf16, tag="u")
                    nc.scalar.activation(
                        u_sb[:, :, :].rearrange("p t h -> p (t h)"),
                        u_ps[:, :],
                        AF.Tanh,
                    )
                    nc.vector.tensor_mul(
                        tmp[:, 16 * half : 16 * (half + 1), :],
                        u_sb,
                        wrep_sb[half][:, :].rearrange("p (t h) -> p t h", h=H),
                    )
                    nc.vector.tensor_reduce(
                        scores[:, 16 * half : 16 * (half + 1)],
                        tmp[:, 16 * half : 16 * (half + 1), :],
                        axis=mybir.AxisListType.X,
                        op=ALU.add,
                    )
                # exp writes straight into the block-diagonal stationary:
                # wsel[32j+k, 8gg+tm, 4tm+j] = exp(scores[32j+k, 8gg+tm]);
                # element offset 256gg + 36tm + j is a linear-strided AP.
                wsel = wsel_bufs[s % NW]
                w_ap = wsel[:, :, :]
                pstride = w_ap.ap[0][0]
                for j in range(4):
                    stripe = bass.AP(
                        tensor=w_ap.tensor,
                        offset=w_ap.offset + 32 * j * pstride + j,
                        ap=[[pstride, 32], [132, 32]],
                    )
                    sc = scores[32 * j : 32 * (j + 1), :]
                    nc.scalar.activation(stripe, sc, AF.Exp)
                # aggregation: the block-diagonal stationary spans all 128
                # nodes (column 4t+j), so 16 fp8 DoubleRow chunk-pair matmuls
                # accumulate ONE [128, 129] PSUM tile at partition offset 0
                # (DoubleRow requires base partition 0; col 128 = Z).
                rzall = mids.tile([128, 1], f32, tag="rz")
                agg_ps = ps_agg.tile([128, NATW], f32, tag="agg")
                for tm2 in range(16):
                    t = 2 * tm2
                    nc.tensor.matmul(
                        agg_ps[:, 0 : D + 1],
                        lhsT=wsel[:, t : t + 2, :],
                        rhs=nb[:, t : t + 2, 0 : D + 1],
                        start=(tm2 == 0),
                        stop=(tm2 == 15),
                        perf_mode=DR,
                    )
                nc.vector.reciprocal(rzall, agg_ps[:, D : D + 1])
                agg_sb = mids.tile([128, D], f32, tag="agg_sb")
                nc.vector.tensor_copy(agg_sb, agg_ps[:, 0:D])
                aggT_ps = ps_small.tile([128, 128], f32, tag="small")
                nc.tensor.transpose(aggT_ps, agg_sb, ident128)
                aggT_sb = mids.tile([128, 128], bf16, tag="aggT_sb")
                nc.vector.tensor_copy(aggT_sb, aggT_ps)
                out2_ps = ps_small.tile([128, O], f32, tag="small")
                nc.tensor.matmul(out2_ps, lhsT=aggT_sb, rhs=wneibb_sb)
                out_sb2 = out_tiles[ip]
                # relu(x) * (1/Z): fused multiply + max on DVE
                nc.scalar.activation(
                    out_sb2[:, g, O : 2 * O], out2_ps, AF.Relu, scale=rzall[:, 0:1]
                )
                if g == GRP - 1:
                    s0 = ip * GRP
                    nc.gpsimd.dma_start(
                        out=out[s0 * 128 : (s0 + GRP) * 128, :].rearrange(
                            "(s p) o -> p s o", s=GRP
                        ),
                        in_=out_sb2,
                    )
                    pair_tiles.pop(ip)
                    w4_tiles.pop(ip)

            for _rep in range(repeat):
                for i in range(npair + 2):
                    if 1 <= i <= npair:
                        node_pair(i - 1)
                    if i >= 2:
                        main_open(i - 2)
                    if i < npair:
                        load_pair(i)
                    if i >= 2:
                        for g in range(GRP):
                            main_path((i - 2) * GRP + g)

    if split_waits:
        _split_multi_waits(nc)
    return nc


def _host_prep(node_feats, neib_feats, W_att1, W_att2, W_node, W_neib, st=ST_FULL):
    """Per-core input dicts with host-side layouts (supertile dim INSIDE the
    partition dim so paired loads are contiguous per partition):
    nbnat[32j+k, s, t, :] = neib row of (node 4t+j, k), col 128 = 1.0 (fp8);
    nbt[d, s, 128t+(32j+k)] = same rows d-major (fp8);
    nodet[:, s, n] = node_feats[s*128+n, :] (bf16)."""
    W1 = np.ascontiguousarray(np.asarray(W_att1, dtype=np.float32))
    W2 = np.asarray(W_att2, dtype=np.float32)
    M2 = (W2.astype(np.float64) @ W2.astype(np.float64).T).astype(np.float32)
    ncn = st * NODES_ST
    n = node_feats.shape[0]
    npad = ncn * NCORES

    from concurrent.futures import ThreadPoolExecutor

    node_pad = np.zeros((npad, D), dtype=BF)
    node_pad[:n] = np.asarray(node_feats, dtype=np.float32).astype(BF)
    neib_q = np.zeros((npad * K, D), dtype=F8)
    nf32 = np.asarray(neib_feats, dtype=np.float32)
    nrows = n * K
    csz = (nrows + 15) // 16
    with ThreadPoolExecutor(16) as tp:
        list(tp.map(
            lambda i: neib_q[i * csz : min((i + 1) * csz, nrows)].__setitem__(
                slice(None), nf32[i * csz : min((i + 1) * csz, nrows)].astype(F8)
            ),
            range(16),
        ))

    w1b = W1.astype(BF)
    w1q = W1.astype(F8)
    m2b = M2.astype(BF)
    wnodeb = np.ascontiguousarray(np.asarray(W_node, np.float32)).astype(BF)
    wneibb = np.ascontiguousarray(np.asarray(W_neib, np.float32)).astype(BF)
    sel4 = np.zeros((4, 128), dtype=BF)
    for j in range(4):
        sel4[j, 32 * j : 32 * (j + 1)] = 1.0

    def prep_core(c):
        nodes = node_pad[c * ncn : (c + 1) * ncn]
        nodet = np.ascontiguousarray(
            nodes.reshape(st, NODES_ST, D).transpose(2, 0, 1)
        )
        x = neib_q[c * ncn * K : (c + 1) * ncn * K].reshape(st, 32, 4, K, D)
        # y[s, 32j+k, t, d] = x[s, t, j, k, d]
        y = np.ascontiguousarray(x.transpose(0, 2, 3, 1, 4)).reshape(
            st, RP, CH, D
        )
        nbnat = np.zeros((RP, st, CH, NATW), dtype=F8)
        nbnat[:, :, :, 0:D] = y.transpose(1, 0, 2, 3)
        nbnat[:, :, :, D] = np.float32(1.0)
        # nbt[d, s, 128t + p] = y[s, p, t, d]
        nbt = np.ascontiguousarray(y.transpose(3, 0, 2, 1)).reshape(
            D, st, CH * RP
        )
        return {
            "nodet": nodet,
            "nbnat": nbnat,
            "nbt": nbt,
            "w1b": w1b,
            "w1q": w1q,
            "m2b": m2b,
            "wnodeb": wnodeb,
            "wneibb": wneibb,
            "sel4": sel4,
        }

    with ThreadPoolExecutor(NCORES) as tp:
        ins = list(tp.map(prep_core, range(NCORES)))
    return ins


def kernel(node_feats, neib_feats, node_ids, neib_ids, W_att1, W_att2, W_node, W_neib):
    from concourse.bass_utils import run_bass_kernel_spmd

    n = node_feats.shape[0]
    in_maps = _host_prep(node_feats, neib_feats, W_att1, W_att2, W_node, W_neib)

    if "nc" not in _module_cache:
        _module_cache["nc"] = build_module(ST_FULL)
    nc = _module_cache["nc"]

    res = run_bass_kernel_spmd(nc, in_maps, core_ids=list(range(NCORES)))
    outs = np.concatenate([res.results[c]["out"] for c in range(NCORES)], axis=0)
    return np.ascontiguousarray(outs[:n].astype(np.float32))
